# revision 71
# baseline (speedup 1.0000x reference)
"""Trainium2 Bass kernel for nn_BrainGeneratorModel (bias-field corrupt + per-sample
separable Gaussian blur), 8-core data parallel.

Sharding: 8 cores = (sample b in 0..3) x (D-half in 0..1). Each core processes a
[108-slab d, H=192, W=192] subvolume (96 interior d' + edge-folded halo) of one
sample.

Per-core pipeline, fully SBUF-resident (no DRAM scratch). Blur+transpose stages
are fused into single data-stationary matmuls: matmul(out, lhsT=DATA, rhs=G)
computes the blur with the data's free dim landing on output partitions, so no
separate PE transpose passes are needed:
  phase A (stream h): bias matmul (K=4) -> exp (ACT) -> x*e (Pool, bf16)
    -> fused D-blur+transpose (stationary = x*e w-slab cols, moving = Gd)
       -> psum [w-slab, d'] -> zw resident tiles, layout (d', h)
  phase B (stream d'): fused W-blur+transpose (stationary = zw h-slab cols,
       moving = Gw slab) -> psum [h-slab, w'] -> zh tiles, layout (d', w')
    -> classic H-blur matmul -> img out (bf16)
All blur matrices are host-built 108x96 slab Toeplitz with replicate-edge
weights folded in, so every matmul contraction is K=108<=128, single pass.

Label remap (pure 32-entry LUT gather) is done on the host.
"""

import sys

for _p in ("/opt/trn_rl_repo",):
    if _p not in sys.path:
        sys.path.insert(0, _p)

import numpy as np
import ml_dtypes

import concourse.bass as bass
import concourse.mybir as mybir
import concourse.bacc as bacc
import concourse.tile as tile
from concourse.bass_utils import run_bass_kernel_spmd

F32 = mybir.dt.float32
BF16 = mybir.dt.bfloat16

B, C, D, H, W = 4, 1, 192, 192, 192
SMALL = 4
BIAS_STD = 0.7
MAX_SIGMA = 3.0
TRUNCATE = 4.0
K = 2 * int(TRUNCATE * MAX_SIGMA) + 1  # 25
P = K // 2  # 12
N_LABELS = 32
TABLE = 128

SLAB = 108          # folded halo slab rows (96 interior + 12, edges folded)
HW = H * W          # 36864
HB = 8              # h rows per phase-A chunk
NHB = H // HB       # 24
CH = HB * W         # 1536 cols per x chunk
QB = 512            # bias/exp psum grain
NQ = CH // QB       # 3
HQ = 4              # h rows per fused-D psum tile (4*96 = 384 cols)
DP = 2              # d' per phase-B step
NG = 96 // DP       # 48
IGB = 4             # g-steps per img staging DMA (8 d' = 1536 cols)

_CACHE = {}


def _lin_weights(n_in, n_out):
    pos = np.linspace(0.0, n_in - 1.0, n_out, dtype=np.float64)
    i0 = np.clip(np.floor(pos).astype(np.int64), 0, n_in - 2)
    f = pos - i0
    Wm = np.zeros((n_out, n_in), np.float64)
    r = np.arange(n_out)
    np.add.at(Wm, (r, i0), 1.0 - f)
    np.add.at(Wm, (r, i0 + 1), f)
    return Wm


def _gauss_kernels(sigma3):
    """sigma3 [3] -> [3, K] kernels exactly as reference."""
    ar = np.arange(K, dtype=np.float64) - K // 2
    out = np.zeros((3, K), np.float64)
    for i, sg in enumerate(sigma3):
        s = max(float(sg), 1e-3)
        g = np.exp(-0.5 * ar * ar / (s * s))
        g = g / g.sum()
        if float(sg) >= 0.01:
            out[i] = g
        else:
            out[i, K // 2] = 1.0
    return out


def _slab_mat(g, out0, slab0, n):
    """[SLAB, 96]: M[src - slab0, j] += g[t] with src = clip(out0+j+t-P, 0, n-1).

    Column j produces absolute output index out0+j from slab rows
    [slab0, slab0+SLAB); replicate-edge taps are folded into edge rows.
    """
    M = np.zeros((SLAB, 96), np.float64)
    for j in range(96):
        for t in range(K):
            src = min(max(out0 + j + t - P, 0), n - 1)
            M[src - slab0, j] += g[t]
    return M


def _build_program():
    nc = bacc.Bacc("TRN2", target_bir_lowering=False, debug=False)

    # ---- external inputs (per core) ----
    xbf_h = nc.dram_tensor("xbf", [SLAB, HW], BF16, kind="ExternalInput")
    sf_h = nc.dram_tensor("sf", [4, HW], BF16, kind="ExternalInput")
    wdt_h = nc.dram_tensor("wdt", [4, SLAB], BF16, kind="ExternalInput")
    gd_h = nc.dram_tensor("gd", [SLAB, 96], BF16, kind="ExternalInput")
    gwa_h = nc.dram_tensor("gwa", [SLAB, 96], BF16, kind="ExternalInput")
    gwb_h = nc.dram_tensor("gwb", [SLAB, 96], BF16, kind="ExternalInput")
    gha_h = nc.dram_tensor("gha", [SLAB, 96], BF16, kind="ExternalInput")
    ghb_h = nc.dram_tensor("ghb", [SLAB, 96], BF16, kind="ExternalInput")

    # ---- external outputs: [d'*192 + w', h'] ----
    img_h = nc.dram_tensor("img", [96 * W, H], BF16, kind="ExternalOutput")

    EXP = mybir.ActivationFunctionType.Exp
    MULT = mybir.AluOpType.mult

    with tile.TileContext(nc) as tc:
        with (
            tc.tile_pool(name="cst", bufs=1) as cst,
            tc.tile_pool(name="res", bufs=1) as res,
            tc.tile_pool(name="xcp", bufs=5) as xcp,
            tc.tile_pool(name="scp", bufs=5) as scp,
            tc.tile_pool(name="ebp", bufs=4) as ebp,
            tc.tile_pool(name="xbp", bufs=4) as xbp,
            tc.tile_pool(name="iap", bufs=3) as iap,
            tc.tile_pool(name="ps", bufs=3, space="PSUM") as psp,
            tc.tile_pool(name="ps2", bufs=5, space="PSUM") as psp2,
        ):
            # ---- constants to SBUF ----
            wdt = cst.tile([4, SLAB], BF16, name="wdt")
            nc.sync.dma_start(wdt[:], wdt_h.ap())
            gd = cst.tile([SLAB, 96], BF16, name="gd")
            nc.sync.dma_start(gd[:], gd_h.ap())
            gwa = cst.tile([SLAB, 96], BF16, name="gwa")
            nc.sync.dma_start(gwa[:], gwa_h.ap())
            gwb = cst.tile([SLAB, 96], BF16, name="gwb")
            nc.sync.dma_start(gwb[:], gwb_h.ap())
            gha = cst.tile([SLAB, 96], BF16, name="gha")
            nc.sync.dma_start(gha[:], gha_h.ap())
            ghb = cst.tile([SLAB, 96], BF16, name="ghb")
            nc.sync.dma_start(ghb[:], ghb_h.ap())

            # ---- resident transposed volumes ----
            zwa = res.tile([SLAB, 96 * H], BF16, name="zwa")  # w 0..108 | (d', h)
            zwb = res.tile([SLAB, 96 * H], BF16, name="zwb")  # w 84..192 | (d', h)
            # zh tiles alias zw: fused_w(g) consumes zw block g right before
            # the zh haul overwrites the same block (WAR dep, range-tracked).
            zha = zwa  # h 0..108 | (d', w')
            zhb = zwb  # h 84..192 | (d', w')

            nzw = 0

            def haul_zw(dst, src):
                nonlocal nzw
                if False:
                    nc.scalar.copy(dst, src)
                else:
                    nc.vector.tensor_copy(dst, src)
                nzw += 1

            # ============ phase A: bias, exp, mult, fused D-blur+T ========
            def front_end(hb):
                c0 = hb * CH
                xc = xcp.tile([SLAB, CH], BF16, tag="xc")
                nc.sync.dma_start(xc[:], xbf_h.ap()[:, c0:c0 + CH])
                sc = scp.tile([4, CH], BF16, tag="sc")
                nc.sync.dma_start(sc[:], sf_h.ap()[:, c0:c0 + CH])

                eb = ebp.tile([SLAB, CH], BF16, tag="eb")
                xb = xbp.tile([SLAB, CH], BF16, tag="xb")
                for q in range(NQ):
                    sl = slice(q * QB, (q + 1) * QB)
                    psb = psp.tile([SLAB, QB], F32, tag="psb")
                    nc.tensor.matmul(psb[:], wdt[:], sc[:, sl], start=True, stop=True)
                    nc.scalar.activation(eb[:, sl], psb[:], EXP)
                    if q % 3 == 0:
                        nc.vector.tensor_tensor(xb[:, sl], xc[:, sl], eb[:, sl], MULT)
                    else:
                        nc.gpsimd.tensor_tensor(xb[:, sl], xc[:, sl], eb[:, sl], MULT)
                return xb

            def fused_d(hb, xb):
                # stationary = xb w-slab cols of one h row; moving = gd.
                # psum [w-slab, d'] batched over HQ h rows.
                for hq in range(HB // HQ):
                    psa = psp2.tile([SLAB, HQ * 96], F32, tag="psa")
                    psb2 = psp2.tile([SLAB, HQ * 96], F32, tag="psa")
                    for t in range(HQ):
                        hl = hq * HQ + t
                        nc.tensor.matmul(
                            psa[:, t * 96:(t + 1) * 96],
                            xb[:, hl * W: hl * W + SLAB], gd[:],
                            start=True, stop=True)
                        nc.tensor.matmul(
                            psb2[:, t * 96:(t + 1) * 96],
                            xb[:, hl * W + 84: hl * W + 192], gd[:],
                            start=True, stop=True)
                    # psum cols (h-local, d') -> zw cols d'*192 + h
                    h0 = hb * HB + hq * HQ
                    for (ps_, zw_) in ((psa, zwa), (psb2, zwb)):
                        haul_zw(
                            zw_[:].rearrange("p (d h) -> p h d", h=H)[:, h0:h0 + HQ, :],
                            ps_[:].rearrange("p (t d) -> p t d", t=HQ),
                        )

            prev = None
            for hb in range(NHB):
                if prev is not None:
                    fused_d(hb - 1, prev)
                prev = front_end(hb)
            fused_d(NHB - 1, prev)

            # ========= phase B: fused W-blur+T, fused H-blur, img out ==
            nzh = 0

            def fused_w(g):
                psta = psp2.tile([SLAB, DP * W], F32, tag="psa")
                pstb = psp2.tile([SLAB, DP * W], F32, tag="psa")
                for dl in range(DP):
                    dcol = (DP * g + dl) * H
                    nc.tensor.matmul(
                        psta[:, dl * W + 0: dl * W + 96],
                        zwa[:, dcol + 0: dcol + SLAB], gwa[:],
                        start=True, stop=True)
                    nc.tensor.matmul(
                        psta[:, dl * W + 96: dl * W + 192],
                        zwb[:, dcol + 0: dcol + SLAB], gwb[:],
                        start=True, stop=True)
                    nc.tensor.matmul(
                        pstb[:, dl * W + 0: dl * W + 96],
                        zwa[:, dcol + 84: dcol + 192], gwa[:],
                        start=True, stop=True)
                    nc.tensor.matmul(
                        pstb[:, dl * W + 96: dl * W + 192],
                        zwb[:, dcol + 84: dcol + 192], gwb[:],
                        start=True, stop=True)
                nonlocal nzh
                gsl = slice(g * DP * W, (g + 1) * DP * W)
                for (z_, ps_) in ((zha, psta), (zhb, pstb)):
                    if nzh % 2 == 1:
                        nc.scalar.copy(z_[:, gsl], ps_[:])
                    else:
                        nc.vector.tensor_copy(z_[:, gsl], ps_[:])
                    nzh += 1

            # fused H: stationary = zh dw-cols (128 at a time); moving = gh
            # slab. psum [dw, (cc, h')]; two chunks per psum tile.
            ist = None

            def fused_h(hp):
                nonlocal ist
                psh = psp.tile([128, 2 * H], F32, tag="psb")
                for cc in range(2):
                    cb = 256 * hp + 128 * cc
                    nc.tensor.matmul(
                        psh[:, cc * H + 0: cc * H + 96],
                        zha[:, cb:cb + 128], gha[:], start=True, stop=True)
                    nc.tensor.matmul(
                        psh[:, cc * H + 96: cc * H + 192],
                        zhb[:, cb:cb + 128], ghb[:], start=True, stop=True)
                if hp % 8 == 0:
                    ist = iap.tile([128, 16 * H], BF16, tag="ist")
                isl = slice((hp % 8) * 2 * H, (hp % 8 + 1) * 2 * H)
                if hp % 2 == 1:
                    nc.scalar.copy(ist[:, isl], psh[:])
                else:
                    nc.vector.tensor_copy(ist[:, isl], psh[:])
                if hp % 8 == 7:
                    # partition-major rows: dram row = p*144 + chunk
                    c0 = (hp // 8) * 16
                    nc.sync.dma_start(
                        bass.AP(img_h, c0 * H,
                                [[144 * H, 128], [H, 16], [1, H]]),
                        ist[:])

            NHP = 96 * W // 256  # 72 fused-H tiles
            hptr = 0
            for g in range(NG):
                fused_w(g)
                while (hptr + 1) * 256 <= 384 * g:
                    fused_h(hptr)
                    hptr += 1
            while hptr < NHP:
                fused_h(hptr)
                hptr += 1
    nc.compile()
    return nc


def _host_prep(x, small_bias, sigma01):
    Wd = _lin_weights(SMALL, D)
    Whm = _lin_weights(SMALL, H)
    Wwm = _lin_weights(SMALL, W)

    in_maps = []
    for c in range(8):
        b, half = c // 2, c % 2
        slab0 = 0 if half == 0 else 84
        d0 = 96 * half

        xs = np.ascontiguousarray(
            np.asarray(x[b, 0, slab0:slab0 + SLAB], np.float32)
        ).reshape(SLAB, HW).astype(ml_dtypes.bfloat16)

        sm = np.asarray(small_bias[b, 0], np.float64) * BIAS_STD
        S = np.einsum("xyz,hy,wz->xhw", sm, Whm, Wwm).reshape(4, HW)

        g3 = _gauss_kernels(np.asarray(sigma01[b], np.float64) * MAX_SIGMA)
        Gd = _slab_mat(g3[0], d0, slab0, D)
        Gwa = _slab_mat(g3[2], 0, 0, W)
        Gwb = _slab_mat(g3[2], 96, 84, W)
        Gha = _slab_mat(g3[1], 0, 0, H)
        Ghb = _slab_mat(g3[1], 96, 84, H)

        in_maps.append({
            "xbf": xs,
            "sf": S.astype(ml_dtypes.bfloat16),
            "wdt": np.ascontiguousarray(Wd[slab0:slab0 + SLAB].T).astype(ml_dtypes.bfloat16),
            "gd": Gd.astype(ml_dtypes.bfloat16),
            "gwa": Gwa.astype(ml_dtypes.bfloat16),
            "gwb": Gwb.astype(ml_dtypes.bfloat16),
            "gha": Gha.astype(ml_dtypes.bfloat16),
            "ghb": Ghb.astype(ml_dtypes.bfloat16),
        })
    return in_maps


def kernel(x, small_bias, sigma01, labels, source_values, dest_values):
    if "nc" not in _CACHE:
        _CACHE["nc"] = _build_program()
    nc = _CACHE["nc"]

    in_maps = _host_prep(x, small_bias, sigma01)
    res = run_bass_kernel_spmd(nc, in_maps, core_ids=list(range(8)))

    img = np.empty((B, C, D, H, W), np.float32)
    for c in range(8):
        b, half = c // 2, c % 2
        d0 = 96 * half
        r = np.asarray(res.results[c]["img"], ml_dtypes.bfloat16)
        # rows are partition-major: row = p*144 + chunk, dw = chunk*128 + p
        r = r.reshape(128, 144, H).transpose(1, 0, 2).reshape(96, W, H)
        # [d', w', h'] -> [d', h', w']
        img[b, 0, d0:d0 + 96] = r.transpose(0, 2, 1).astype(np.float32)

    mapping = np.zeros(TABLE, np.int32)
    mapping[np.asarray(source_values, np.int64)] = (
        np.asarray(dest_values, np.int64).astype(np.int32))
    labels_out = mapping[np.asarray(labels, np.int64)]
    return img, labels_out


# revision 96
# speedup vs baseline: 1.0241x; 1.0241x over previous
"""Trainium2 Bass kernel for nn_BrainGeneratorModel (bias-field corrupt + per-sample
separable Gaussian blur), 8-core data parallel.

Sharding: 8 cores = (sample b in 0..3) x (D-half in 0..1). Each core processes a
[108-slab d, H=192, W=192] subvolume (96 interior d' + edge-folded halo) of one
sample.

Per-core pipeline, fully SBUF-resident (no DRAM scratch). Blur+transpose stages
are fused into single data-stationary matmuls: matmul(out, lhsT=DATA, rhs=G)
computes the blur with the data's free dim landing on output partitions, so no
separate PE transpose passes are needed:
  phase A (stream h): bias matmul (K=4) -> exp (ACT) -> x*e (Pool, bf16)
    -> fused D-blur+transpose (stationary = x*e w-slab cols, moving = Gd)
       -> psum [w-slab, d'] -> zw resident tiles, layout (d', h)
  phase B (stream d'): fused W-blur+transpose (stationary = zw h-slab cols,
       moving = Gw slab) -> psum [h-slab, w'] -> zh tiles, layout (d', w')
    -> classic H-blur matmul -> img out (bf16)
All blur matrices are host-built 108x96 slab Toeplitz with replicate-edge
weights folded in, so every matmul contraction is K=108<=128, single pass.

Label remap (pure 32-entry LUT gather) is done on the host.
"""

import sys

for _p in ("/opt/trn_rl_repo",):
    if _p not in sys.path:
        sys.path.insert(0, _p)

import numpy as np
import ml_dtypes

import concourse.bass as bass
import concourse.mybir as mybir
import concourse.bacc as bacc
import concourse.tile as tile
from concourse.bass_utils import run_bass_kernel_spmd

F32 = mybir.dt.float32
BF16 = mybir.dt.bfloat16

B, C, D, H, W = 4, 1, 192, 192, 192
SMALL = 4
BIAS_STD = 0.7
MAX_SIGMA = 3.0
TRUNCATE = 4.0
K = 2 * int(TRUNCATE * MAX_SIGMA) + 1  # 25
P = K // 2  # 12
N_LABELS = 32
TABLE = 128

SLAB = 108          # folded halo slab rows (96 interior + 12, edges folded)
HW = H * W          # 36864
HB = 8              # h rows per phase-A chunk
NHB = H // HB       # 24
CH = HB * W         # 1536 cols per x chunk
QB = 512            # bias/exp psum grain
NQ = CH // QB       # 3
HQ = 4              # h rows per fused-D psum tile (4*96 = 384 cols)
DP = 2              # d' per phase-B step
NG = 96 // DP       # 48
IGB = 4             # g-steps per img staging DMA (8 d' = 1536 cols)

_CACHE = {}


def _lin_weights(n_in, n_out):
    pos = np.linspace(0.0, n_in - 1.0, n_out, dtype=np.float64)
    i0 = np.clip(np.floor(pos).astype(np.int64), 0, n_in - 2)
    f = pos - i0
    Wm = np.zeros((n_out, n_in), np.float64)
    r = np.arange(n_out)
    np.add.at(Wm, (r, i0), 1.0 - f)
    np.add.at(Wm, (r, i0 + 1), f)
    return Wm


def _gauss_kernels(sigma3):
    """sigma3 [3] -> [3, K] kernels exactly as reference."""
    ar = np.arange(K, dtype=np.float64) - K // 2
    out = np.zeros((3, K), np.float64)
    for i, sg in enumerate(sigma3):
        s = max(float(sg), 1e-3)
        g = np.exp(-0.5 * ar * ar / (s * s))
        g = g / g.sum()
        if float(sg) >= 0.01:
            out[i] = g
        else:
            out[i, K // 2] = 1.0
    return out


def _slab_mat(g, out0, slab0, n):
    """[SLAB, 96]: M[src - slab0, j] += g[t] with src = clip(out0+j+t-P, 0, n-1).

    Column j produces absolute output index out0+j from slab rows
    [slab0, slab0+SLAB); replicate-edge taps are folded into edge rows.
    """
    M = np.zeros((SLAB, 96), np.float64)
    for j in range(96):
        for t in range(K):
            src = min(max(out0 + j + t - P, 0), n - 1)
            M[src - slab0, j] += g[t]
    return M


def _build_program():
    nc = bacc.Bacc("TRN2", target_bir_lowering=False, debug=False)

    # ---- external inputs (per core) ----
    xbf_h = nc.dram_tensor("xbf", [SLAB, HW], BF16, kind="ExternalInput")
    sf_h = nc.dram_tensor("sf", [4, HW], BF16, kind="ExternalInput")
    wdt_h = nc.dram_tensor("wdt", [4, SLAB], BF16, kind="ExternalInput")
    gd_h = nc.dram_tensor("gd", [SLAB, 96], BF16, kind="ExternalInput")
    gwa_h = nc.dram_tensor("gwa", [SLAB, 96], BF16, kind="ExternalInput")
    gwb_h = nc.dram_tensor("gwb", [SLAB, 96], BF16, kind="ExternalInput")
    gha_h = nc.dram_tensor("gha", [SLAB, 96], BF16, kind="ExternalInput")
    ghb_h = nc.dram_tensor("ghb", [SLAB, 96], BF16, kind="ExternalInput")

    # ---- external outputs: [d'*192 + w', h'] ----
    img_h = nc.dram_tensor("img", [96 * W, H], BF16, kind="ExternalOutput")

    EXP = mybir.ActivationFunctionType.Exp
    MULT = mybir.AluOpType.mult

    with tile.TileContext(nc) as tc:
        with (
            tc.tile_pool(name="cst", bufs=1) as cst,
            tc.tile_pool(name="res", bufs=1) as res,
            tc.tile_pool(name="xcp", bufs=5) as xcp,
            tc.tile_pool(name="scp", bufs=5) as scp,
            tc.tile_pool(name="ebp", bufs=4) as ebp,
            tc.tile_pool(name="xbp", bufs=4) as xbp,
            tc.tile_pool(name="iap", bufs=3) as iap,
            tc.tile_pool(name="ps", bufs=3, space="PSUM") as psp,
            tc.tile_pool(name="ps2", bufs=5, space="PSUM") as psp2,
        ):
            # ---- constants to SBUF ----
            wdt = cst.tile([4, SLAB], BF16, name="wdt")
            nc.sync.dma_start(wdt[:], wdt_h.ap())
            gd = cst.tile([SLAB, 96], BF16, name="gd")
            nc.sync.dma_start(gd[:], gd_h.ap())
            gwa = cst.tile([SLAB, 96], BF16, name="gwa")
            gwb = cst.tile([SLAB, 96], BF16, name="gwb")
            gha = cst.tile([SLAB, 96], BF16, name="gha")
            ghb = cst.tile([SLAB, 96], BF16, name="ghb")

            # ---- full S operand (bias matmul rhs), loaded once ----
            scf = res.tile([4, HW], BF16, name="scf")
            nc.sync.dma_start(scf[:, 0:CH * 2], sf_h.ap()[:, 0:CH * 2])

            # ---- resident transposed volumes ----
            zwa = res.tile([SLAB, 96 * H], BF16, name="zwa")  # w 0..108 | (d', h)
            zwb = res.tile([SLAB, 96 * H], BF16, name="zwb")  # w 84..192 | (d', h)
            # zh tiles alias zw: fused_w(g) consumes zw block g right before
            # the zh haul overwrites the same block (WAR dep, range-tracked).
            zha = zwa  # h 0..108 | (d', w')
            zhb = zwb  # h 84..192 | (d', w')

            nzw = 0

            def haul_zw(dst, src):
                nonlocal nzw
                if False:
                    nc.scalar.copy(dst, src)
                else:
                    nc.vector.tensor_copy(dst, src)
                nzw += 1

            # ============ phase A: bias, exp, mult, fused D-blur+T ========
            def front_end(hb):
                c0 = hb * CH
                xc = xcp.tile([SLAB, CH], BF16, tag="xc")
                nc.sync.dma_start(xc[:], xbf_h.ap()[:, c0:c0 + CH])
                eb = ebp.tile([SLAB, CH], BF16, tag="eb")
                xb = xbp.tile([SLAB, CH], BF16, tag="xb")
                for q in range(NQ):
                    sl = slice(q * QB, (q + 1) * QB)
                    psb = psp.tile([SLAB, QB], F32, tag="psb")
                    nc.tensor.matmul(psb[:], wdt[:], scf[:, c0 + q * QB: c0 + (q + 1) * QB],
                                     start=True, stop=True)
                    nc.scalar.activation(eb[:, sl], psb[:], EXP)
                    if q % 3 == 0:
                        nc.vector.tensor_tensor(xb[:, sl], xc[:, sl], eb[:, sl], MULT)
                    else:
                        nc.gpsimd.tensor_tensor(xb[:, sl], xc[:, sl], eb[:, sl], MULT)
                return xb

            def fused_d(hb, xb):
                # stationary = xb w-slab cols of one h row; moving = gd.
                # psum [w-slab, d'] batched over HQ h rows.
                for hq in range(HB // HQ):
                    psa = psp2.tile([SLAB, HQ * 96], F32, tag="psa")
                    psb2 = psp2.tile([SLAB, HQ * 96], F32, tag="psa")
                    for t in range(HQ):
                        hl = hq * HQ + t
                        nc.tensor.matmul(
                            psa[:, t * 96:(t + 1) * 96],
                            xb[:, hl * W: hl * W + SLAB], gd[:],
                            start=True, stop=True)
                        nc.tensor.matmul(
                            psb2[:, t * 96:(t + 1) * 96],
                            xb[:, hl * W + 84: hl * W + 192], gd[:],
                            start=True, stop=True)
                    # psum cols (h-local, d') -> zw cols d'*192 + h
                    h0 = hb * HB + hq * HQ
                    for (ps_, zw_) in ((psa, zwa), (psb2, zwb)):
                        haul_zw(
                            zw_[:].rearrange("p (d h) -> p h d", h=H)[:, h0:h0 + HQ, :],
                            ps_[:].rearrange("p (t d) -> p t d", t=HQ),
                        )

            prev = None
            for hb in range(NHB):
                if prev is not None:
                    fused_d(hb - 1, prev)
                prev = front_end(hb)
                if hb == 0:
                    nc.sync.dma_start(scf[:, CH * 2:HW // 2],
                                      sf_h.ap()[:, CH * 2:HW // 2])
                if hb == 10:
                    nc.sync.dma_start(scf[:, HW // 2:HW],
                                      sf_h.ap()[:, HW // 2:HW])
                if hb == 23:
                    # phase-B constants: load after the pipeline is primed
                    nc.sync.dma_start(gwa[:], gwa_h.ap())
                    nc.sync.dma_start(gwb[:], gwb_h.ap())
                    nc.sync.dma_start(gha[:], gha_h.ap())
                    nc.sync.dma_start(ghb[:], ghb_h.ap())
            fused_d(NHB - 1, prev)

            # ========= phase B: fused W-blur+T, fused H-blur, img out ==
            nzh = 0

            def fused_w(g):
                psta = psp2.tile([SLAB, DP * W], F32, tag="psa")
                pstb = psp2.tile([SLAB, DP * W], F32, tag="psa")
                for dl in range(DP):
                    dcol = (DP * g + dl) * H
                    nc.tensor.matmul(
                        psta[:, dl * W + 0: dl * W + 96],
                        zwa[:, dcol + 0: dcol + SLAB], gwa[:],
                        start=True, stop=True)
                    nc.tensor.matmul(
                        psta[:, dl * W + 96: dl * W + 192],
                        zwb[:, dcol + 0: dcol + SLAB], gwb[:],
                        start=True, stop=True)
                    nc.tensor.matmul(
                        pstb[:, dl * W + 0: dl * W + 96],
                        zwa[:, dcol + 84: dcol + 192], gwa[:],
                        start=True, stop=True)
                    nc.tensor.matmul(
                        pstb[:, dl * W + 96: dl * W + 192],
                        zwb[:, dcol + 84: dcol + 192], gwb[:],
                        start=True, stop=True)
                nonlocal nzh
                gsl = slice(g * DP * W, (g + 1) * DP * W)
                for (z_, ps_) in ((zha, psta), (zhb, pstb)):
                    if nzh % 2 == 1:
                        nc.scalar.copy(z_[:, gsl], ps_[:])
                    else:
                        nc.vector.tensor_copy(z_[:, gsl], ps_[:])
                    nzh += 1

            # fused H: stationary = zh dw-cols (128 at a time); moving = gh
            # slab. psum [dw, (cc, h')]; two chunks per psum tile.
            ist = None

            def fused_h(hp):
                nonlocal ist
                psh = psp.tile([128, 2 * H], F32, tag="psb")
                for cc in range(2):
                    cb = 256 * hp + 128 * cc
                    nc.tensor.matmul(
                        psh[:, cc * H + 0: cc * H + 96],
                        zha[:, cb:cb + 128], gha[:], start=True, stop=True)
                    nc.tensor.matmul(
                        psh[:, cc * H + 96: cc * H + 192],
                        zhb[:, cb:cb + 128], ghb[:], start=True, stop=True)
                if hp % 8 == 0:
                    ist = iap.tile([128, 16 * H], BF16, tag="ist")
                isl = slice((hp % 8) * 2 * H, (hp % 8 + 1) * 2 * H)
                if hp % 2 == 1:
                    nc.scalar.copy(ist[:, isl], psh[:])
                else:
                    nc.vector.tensor_copy(ist[:, isl], psh[:])
                if hp % 8 == 7:
                    # partition-major rows: dram row = p*144 + chunk
                    c0 = (hp // 8) * 16
                    nc.sync.dma_start(
                        bass.AP(img_h, c0 * H,
                                [[144 * H, 128], [H, 16], [1, H]]),
                        ist[:])

            NHP = 96 * W // 256  # 72 fused-H tiles
            hptr = 0
            for g in range(NG):
                fused_w(g)
                while (hptr + 1) * 256 <= 384 * g:
                    fused_h(hptr)
                    hptr += 1
            while hptr < NHP:
                fused_h(hptr)
                hptr += 1
    nc.compile()
    return nc


def _host_prep(x, small_bias, sigma01):
    Wd = _lin_weights(SMALL, D)
    Whm = _lin_weights(SMALL, H)
    Wwm = _lin_weights(SMALL, W)

    in_maps = []
    for c in range(8):
        b, half = c // 2, c % 2
        slab0 = 0 if half == 0 else 84
        d0 = 96 * half

        xs = np.ascontiguousarray(
            np.asarray(x[b, 0, slab0:slab0 + SLAB], np.float32)
        ).reshape(SLAB, HW).astype(ml_dtypes.bfloat16)

        sm = np.asarray(small_bias[b, 0], np.float64) * BIAS_STD
        S = np.einsum("xyz,hy,wz->xhw", sm, Whm, Wwm).reshape(4, HW)

        g3 = _gauss_kernels(np.asarray(sigma01[b], np.float64) * MAX_SIGMA)
        Gd = _slab_mat(g3[0], d0, slab0, D)
        Gwa = _slab_mat(g3[2], 0, 0, W)
        Gwb = _slab_mat(g3[2], 96, 84, W)
        Gha = _slab_mat(g3[1], 0, 0, H)
        Ghb = _slab_mat(g3[1], 96, 84, H)

        in_maps.append({
            "xbf": xs,
            "sf": S.astype(ml_dtypes.bfloat16),
            "wdt": np.ascontiguousarray(Wd[slab0:slab0 + SLAB].T).astype(ml_dtypes.bfloat16),
            "gd": Gd.astype(ml_dtypes.bfloat16),
            "gwa": Gwa.astype(ml_dtypes.bfloat16),
            "gwb": Gwb.astype(ml_dtypes.bfloat16),
            "gha": Gha.astype(ml_dtypes.bfloat16),
            "ghb": Ghb.astype(ml_dtypes.bfloat16),
        })
    return in_maps


def kernel(x, small_bias, sigma01, labels, source_values, dest_values):
    if "nc" not in _CACHE:
        _CACHE["nc"] = _build_program()
    nc = _CACHE["nc"]

    in_maps = _host_prep(x, small_bias, sigma01)
    res = run_bass_kernel_spmd(nc, in_maps, core_ids=list(range(8)))

    img = np.empty((B, C, D, H, W), np.float32)
    for c in range(8):
        b, half = c // 2, c % 2
        d0 = 96 * half
        r = np.asarray(res.results[c]["img"], ml_dtypes.bfloat16)
        # rows are partition-major: row = p*144 + chunk, dw = chunk*128 + p
        r = r.reshape(128, 144, H).transpose(1, 0, 2).reshape(96, W, H)
        # [d', w', h'] -> [d', h', w']
        img[b, 0, d0:d0 + 96] = r.transpose(0, 2, 1).astype(np.float32)

    mapping = np.zeros(TABLE, np.int32)
    mapping[np.asarray(source_values, np.int64)] = (
        np.asarray(dest_values, np.int64).astype(np.int32))
    labels_out = mapping[np.asarray(labels, np.int64)]
    return img, labels_out


# revision 111
# speedup vs baseline: 1.0627x; 1.0377x over previous
"""Trainium2 Bass kernel for nn_BrainGeneratorModel (bias-field corrupt + per-sample
separable Gaussian blur), 8-core data parallel.

Sharding: 8 cores = (sample b in 0..3) x (D-half in 0..1). Each core processes a
[108-slab d, H=192, W=192] subvolume (96 interior d' + edge-folded halo) of one
sample.

Per-core pipeline, fully SBUF-resident (no DRAM scratch). Blur+transpose stages
are fused into single data-stationary matmuls: matmul(out, lhsT=DATA, rhs=G)
computes the blur with the data's free dim landing on output partitions, so no
separate PE transpose passes are needed:
  phase A (stream h): bias matmul (K=4) -> exp (ACT) -> x*e (Pool, bf16)
    -> fused D-blur+transpose (stationary = x*e w-slab cols, moving = Gd)
       -> psum [w-slab, d'] -> zw resident tiles, layout (d', h)
  phase B (stream d'): fused W-blur+transpose (stationary = zw h-slab cols,
       moving = Gw slab) -> psum [h-slab, w'] -> zh tiles, layout (d', w')
    -> classic H-blur matmul -> img out (bf16)
All blur matrices are host-built 108x96 slab Toeplitz with replicate-edge
weights folded in, so every matmul contraction is K=108<=128, single pass.

Label remap (pure 32-entry LUT gather) is done on the host.
"""

import sys

for _p in ("/opt/trn_rl_repo",):
    if _p not in sys.path:
        sys.path.insert(0, _p)

import numpy as np
import ml_dtypes

import concourse.bass as bass
import concourse.mybir as mybir
import concourse.bacc as bacc
import concourse.tile as tile
from concourse.bass_utils import run_bass_kernel_spmd

F32 = mybir.dt.float32
BF16 = mybir.dt.bfloat16

B, C, D, H, W = 4, 1, 192, 192, 192
SMALL = 4
BIAS_STD = 0.7
MAX_SIGMA = 3.0
TRUNCATE = 4.0
K = 2 * int(TRUNCATE * MAX_SIGMA) + 1  # 25
P = K // 2  # 12
N_LABELS = 32
TABLE = 128

SLAB = 108          # folded halo slab rows (96 interior + 12, edges folded)
HW = H * W          # 36864
HB = 8              # h rows per phase-A chunk
NHB = H // HB       # 24
CH = HB * W         # 1536 cols per x chunk
QB = 512            # bias/exp psum grain
NQ = CH // QB       # 3
HQ = 4              # h rows per fused-D psum tile (4*96 = 384 cols)
DP = 2              # d' per phase-B step
NG = 96 // DP       # 48
IGB = 4             # g-steps per img staging DMA (8 d' = 1536 cols)

_CACHE = {}


def _lin_weights(n_in, n_out):
    pos = np.linspace(0.0, n_in - 1.0, n_out, dtype=np.float64)
    i0 = np.clip(np.floor(pos).astype(np.int64), 0, n_in - 2)
    f = pos - i0
    Wm = np.zeros((n_out, n_in), np.float64)
    r = np.arange(n_out)
    np.add.at(Wm, (r, i0), 1.0 - f)
    np.add.at(Wm, (r, i0 + 1), f)
    return Wm


def _gauss_kernels(sigma3):
    """sigma3 [3] -> [3, K] kernels exactly as reference."""
    ar = np.arange(K, dtype=np.float64) - K // 2
    out = np.zeros((3, K), np.float64)
    for i, sg in enumerate(sigma3):
        s = max(float(sg), 1e-3)
        g = np.exp(-0.5 * ar * ar / (s * s))
        g = g / g.sum()
        if float(sg) >= 0.01:
            out[i] = g
        else:
            out[i, K // 2] = 1.0
    return out


def _slab_mat(g, out0, slab0, n):
    """[SLAB, 96]: M[src - slab0, j] += g[t] with src = clip(out0+j+t-P, 0, n-1).

    Column j produces absolute output index out0+j from slab rows
    [slab0, slab0+SLAB); replicate-edge taps are folded into edge rows.
    """
    M = np.zeros((SLAB, 96), np.float64)
    for j in range(96):
        for t in range(K):
            src = min(max(out0 + j + t - P, 0), n - 1)
            M[src - slab0, j] += g[t]
    return M


def _build_program():
    nc = bacc.Bacc("TRN2", target_bir_lowering=False, debug=False)

    # ---- external inputs (per core) ----
    xbf_h = nc.dram_tensor("xbf", [SLAB, HW], BF16, kind="ExternalInput")
    sf_h = nc.dram_tensor("sf", [4, HW], BF16, kind="ExternalInput")
    wdt_h = nc.dram_tensor("wdt", [4, SLAB], BF16, kind="ExternalInput")
    gd_h = nc.dram_tensor("gd", [SLAB, 96], BF16, kind="ExternalInput")
    gwa_h = nc.dram_tensor("gwa", [SLAB, 96], BF16, kind="ExternalInput")
    gwb_h = nc.dram_tensor("gwb", [SLAB, 96], BF16, kind="ExternalInput")
    gha_h = nc.dram_tensor("gha", [SLAB, 96], BF16, kind="ExternalInput")
    ghb_h = nc.dram_tensor("ghb", [SLAB, 96], BF16, kind="ExternalInput")

    # ---- external outputs: [d'*192 + w', h'] ----
    img_h = nc.dram_tensor("img", [96 * W, H], BF16, kind="ExternalOutput")

    EXP = mybir.ActivationFunctionType.Exp
    MULT = mybir.AluOpType.mult

    with tile.TileContext(nc) as tc:
        with (
            tc.tile_pool(name="cst", bufs=1) as cst,
            tc.tile_pool(name="res", bufs=1) as res,
            tc.tile_pool(name="xcp", bufs=5) as xcp,
            tc.tile_pool(name="scp", bufs=5) as scp,
            tc.tile_pool(name="ebp", bufs=4) as ebp,
            tc.tile_pool(name="xbp", bufs=4) as xbp,
            tc.tile_pool(name="iap", bufs=3) as iap,
            tc.tile_pool(name="ps", bufs=3, space="PSUM") as psp,
            tc.tile_pool(name="ps2", bufs=5, space="PSUM") as psp2,
        ):
            # ---- constants to SBUF ----
            wdt = cst.tile([4, SLAB], BF16, name="wdt")
            nc.sync.dma_start(wdt[:], wdt_h.ap())
            gd = cst.tile([SLAB, 96], BF16, name="gd")
            nc.sync.dma_start(gd[:], gd_h.ap())
            gwa = cst.tile([SLAB, 96], BF16, name="gwa")
            gwb = cst.tile([SLAB, 96], BF16, name="gwb")
            gha = cst.tile([SLAB, 96], BF16, name="gha")
            ghb = cst.tile([SLAB, 96], BF16, name="ghb")

            # ---- full S operand (bias matmul rhs), loaded once ----
            scf = res.tile([4, HW], BF16, name="scf")
            nc.sync.dma_start(scf[:, 0:CH * 2], sf_h.ap()[:, 0:CH * 2])

            # ---- resident transposed volumes ----
            zwa = res.tile([SLAB, 96 * H], BF16, name="zwa")  # w 0..108 | (d', h)
            zwb = res.tile([SLAB, 96 * H], BF16, name="zwb")  # w 84..192 | (d', h)
            # zh tiles alias zw: fused_w(g) consumes zw block g right before
            # the zh haul overwrites the same block (WAR dep, range-tracked).
            zha = zwa  # h 0..108 | (d', w')
            zhb = zwb  # h 84..192 | (d', w')

            nzw = 0

            def haul_zw(dst, src):
                nonlocal nzw
                if False:
                    nc.scalar.copy(dst, src)
                else:
                    nc.vector.tensor_copy(dst, src)
                nzw += 1

            # ============ phase A: bias, exp, mult, fused D-blur+T ========
            def front_end(hb):
                c0 = hb * CH
                xc = xcp.tile([SLAB, CH], BF16, tag="xc")
                nc.sync.dma_start(xc[:], xbf_h.ap()[:, c0:c0 + CH])
                eb = ebp.tile([SLAB, CH], BF16, tag="eb")
                xb = xbp.tile([SLAB, CH], BF16, tag="xb")
                for q in range(NQ):
                    sl = slice(q * QB, (q + 1) * QB)
                    psb = psp.tile([SLAB, QB], F32, tag="psb")
                    nc.tensor.matmul(psb[:], wdt[:], scf[:, c0 + q * QB: c0 + (q + 1) * QB],
                                     start=True, stop=True)
                    nc.scalar.activation(eb[:, sl], psb[:], EXP)
                    if q % 3 == 0 or (hb >= 22 and q == 1):
                        nc.vector.tensor_tensor(xb[:, sl], xc[:, sl], eb[:, sl], MULT)
                    else:
                        nc.gpsimd.tensor_tensor(xb[:, sl], xc[:, sl], eb[:, sl], MULT)
                return xb

            def fused_d(hb, xb):
                # stationary = xb w-slab cols of one h row; moving = gd.
                # psum [w-slab, d'] batched over HQ h rows.
                for hq in range(HB // HQ):
                    psa = psp2.tile([SLAB, HQ * 96], F32, tag="psa")
                    psb2 = psp2.tile([SLAB, HQ * 96], F32, tag="psa")
                    for t in range(HQ):
                        hl = hq * HQ + t
                        nc.tensor.matmul(
                            psa[:, t * 96:(t + 1) * 96],
                            xb[:, hl * W: hl * W + SLAB], gd[:],
                            start=True, stop=True)
                        nc.tensor.matmul(
                            psb2[:, t * 96:(t + 1) * 96],
                            xb[:, hl * W + 84: hl * W + 192], gd[:],
                            start=True, stop=True)
                    # psum cols (h-local, d') -> zw cols d'*192 + h
                    # late chunks: ACT's exp queue has drained, give it the
                    # second haul of each pair
                    h0 = hb * HB + hq * HQ
                    for i, (ps_, zw_) in enumerate(((psa, zwa), (psb2, zwb))):
                        dst = zw_[:].rearrange("p (d h) -> p h d", h=H)[:, h0:h0 + HQ, :]
                        s_ = ps_[:].rearrange("p (t d) -> p t d", t=HQ)
                        if hb >= 19 and i == 1:
                            nc.scalar.copy(dst, s_)
                        else:
                            nc.vector.tensor_copy(dst, s_)

            prev = None
            for hb in range(NHB):
                if prev is not None:
                    fused_d(hb - 1, prev)
                prev = front_end(hb)
                if hb == 0:
                    nc.sync.dma_start(scf[:, CH * 2:HW // 2],
                                      sf_h.ap()[:, CH * 2:HW // 2])
                if hb == 10:
                    nc.sync.dma_start(scf[:, HW // 2:HW],
                                      sf_h.ap()[:, HW // 2:HW])
                if hb == 23:
                    # phase-B constants: load after the pipeline is primed
                    nc.sync.dma_start(gwa[:], gwa_h.ap())
                    nc.sync.dma_start(gwb[:], gwb_h.ap())
                    nc.sync.dma_start(gha[:], gha_h.ap())
                    nc.sync.dma_start(ghb[:], ghb_h.ap())
            fused_d(NHB - 1, prev)

            # ========= phase B: fused W-blur+T, fused H-blur, img out ==
            nzh = 0

            def fused_w(g):
                psta = psp2.tile([SLAB, DP * W], F32, tag="psa")
                pstb = psp2.tile([SLAB, DP * W], F32, tag="psa")
                for dl in range(DP):
                    dcol = (DP * g + dl) * H
                    nc.tensor.matmul(
                        psta[:, dl * W + 0: dl * W + 96],
                        zwa[:, dcol + 0: dcol + SLAB], gwa[:],
                        start=True, stop=True)
                    nc.tensor.matmul(
                        psta[:, dl * W + 96: dl * W + 192],
                        zwb[:, dcol + 0: dcol + SLAB], gwb[:],
                        start=True, stop=True)
                    nc.tensor.matmul(
                        pstb[:, dl * W + 0: dl * W + 96],
                        zwa[:, dcol + 84: dcol + 192], gwa[:],
                        start=True, stop=True)
                    nc.tensor.matmul(
                        pstb[:, dl * W + 96: dl * W + 192],
                        zwb[:, dcol + 84: dcol + 192], gwb[:],
                        start=True, stop=True)
                nonlocal nzh
                gsl = slice(g * DP * W, (g + 1) * DP * W)
                for (z_, ps_) in ((zha, psta), (zhb, pstb)):
                    if nzh % 2 == 1:
                        nc.scalar.copy(z_[:, gsl], ps_[:])
                    else:
                        nc.vector.tensor_copy(z_[:, gsl], ps_[:])
                    nzh += 1

            # fused H: stationary = zh dw-cols (128 at a time); moving = gh
            # slab. psum [dw, (cc, h')]; two chunks per psum tile.
            ist = None

            def fused_h(hp):
                nonlocal ist
                psh = psp.tile([128, 2 * H], F32, tag="psb")
                for cc in range(2):
                    cb = 256 * hp + 128 * cc
                    nc.tensor.matmul(
                        psh[:, cc * H + 0: cc * H + 96],
                        zha[:, cb:cb + 128], gha[:], start=True, stop=True)
                    nc.tensor.matmul(
                        psh[:, cc * H + 96: cc * H + 192],
                        zhb[:, cb:cb + 128], ghb[:], start=True, stop=True)
                if hp % 8 == 0:
                    ist = iap.tile([128, 16 * H], BF16, tag="ist")
                isl = slice((hp % 8) * 2 * H, (hp % 8 + 1) * 2 * H)
                if hp % 2 == 1:
                    nc.scalar.copy(ist[:, isl], psh[:])
                else:
                    nc.vector.tensor_copy(ist[:, isl], psh[:])
                if hp % 8 == 7:
                    # partition-major rows: dram row = p*144 + chunk
                    c0 = (hp // 8) * 16
                    nc.sync.dma_start(
                        bass.AP(img_h, c0 * H,
                                [[144 * H, 128], [H, 16], [1, H]]),
                        ist[:])

            NHP = 96 * W // 256  # 72 fused-H tiles
            hptr = 0
            for g in range(NG):
                fused_w(g)
                while (hptr + 1) * 256 <= 384 * g:
                    fused_h(hptr)
                    hptr += 1
            while hptr < NHP:
                fused_h(hptr)
                hptr += 1
    nc.compile()
    return nc


def _host_prep(x, small_bias, sigma01):
    Wd = _lin_weights(SMALL, D)
    Whm = _lin_weights(SMALL, H)
    Wwm = _lin_weights(SMALL, W)

    in_maps = []
    for c in range(8):
        b, half = c // 2, c % 2
        slab0 = 0 if half == 0 else 84
        d0 = 96 * half

        xs = np.ascontiguousarray(
            np.asarray(x[b, 0, slab0:slab0 + SLAB], np.float32)
        ).reshape(SLAB, HW).astype(ml_dtypes.bfloat16)

        sm = np.asarray(small_bias[b, 0], np.float64) * BIAS_STD
        S = np.einsum("xyz,hy,wz->xhw", sm, Whm, Wwm).reshape(4, HW)

        g3 = _gauss_kernels(np.asarray(sigma01[b], np.float64) * MAX_SIGMA)
        Gd = _slab_mat(g3[0], d0, slab0, D)
        Gwa = _slab_mat(g3[2], 0, 0, W)
        Gwb = _slab_mat(g3[2], 96, 84, W)
        Gha = _slab_mat(g3[1], 0, 0, H)
        Ghb = _slab_mat(g3[1], 96, 84, H)

        in_maps.append({
            "xbf": xs,
            "sf": S.astype(ml_dtypes.bfloat16),
            "wdt": np.ascontiguousarray(Wd[slab0:slab0 + SLAB].T).astype(ml_dtypes.bfloat16),
            "gd": Gd.astype(ml_dtypes.bfloat16),
            "gwa": Gwa.astype(ml_dtypes.bfloat16),
            "gwb": Gwb.astype(ml_dtypes.bfloat16),
            "gha": Gha.astype(ml_dtypes.bfloat16),
            "ghb": Ghb.astype(ml_dtypes.bfloat16),
        })
    return in_maps


def kernel(x, small_bias, sigma01, labels, source_values, dest_values):
    if "nc" not in _CACHE:
        _CACHE["nc"] = _build_program()
    nc = _CACHE["nc"]

    in_maps = _host_prep(x, small_bias, sigma01)
    res = run_bass_kernel_spmd(nc, in_maps, core_ids=list(range(8)))

    img = np.empty((B, C, D, H, W), np.float32)
    for c in range(8):
        b, half = c // 2, c % 2
        d0 = 96 * half
        r = np.asarray(res.results[c]["img"], ml_dtypes.bfloat16)
        # rows are partition-major: row = p*144 + chunk, dw = chunk*128 + p
        r = r.reshape(128, 144, H).transpose(1, 0, 2).reshape(96, W, H)
        # [d', w', h'] -> [d', h', w']
        img[b, 0, d0:d0 + 96] = r.transpose(0, 2, 1).astype(np.float32)

    mapping = np.zeros(TABLE, np.int32)
    mapping[np.asarray(source_values, np.int64)] = (
        np.asarray(dest_values, np.int64).astype(np.int32))
    labels_out = mapping[np.asarray(labels, np.int64)]
    return img, labels_out


# revision 118
# speedup vs baseline: 1.0638x; 1.0011x over previous
"""Trainium2 Bass kernel for nn_BrainGeneratorModel (bias-field corrupt + per-sample
separable Gaussian blur), 8-core data parallel.

Sharding: 8 cores = (sample b in 0..3) x (D-half in 0..1). Each core processes a
[108-slab d, H=192, W=192] subvolume (96 interior d' + edge-folded halo) of one
sample.

Per-core pipeline, fully SBUF-resident (no DRAM scratch). Blur+transpose stages
are fused into single data-stationary matmuls: matmul(out, lhsT=DATA, rhs=G)
computes the blur with the data's free dim landing on output partitions, so no
separate PE transpose passes are needed:
  phase A (stream h): bias matmul (K=4) -> exp (ACT) -> x*e (Pool, bf16)
    -> fused D-blur+transpose (stationary = x*e w-slab cols, moving = Gd)
       -> psum [w-slab, d'] -> zw resident tiles, layout (d', h)
  phase B (stream d'): fused W-blur+transpose (stationary = zw h-slab cols,
       moving = Gw slab) -> psum [h-slab, w'] -> zh tiles, layout (d', w')
    -> classic H-blur matmul -> img out (bf16)
All blur matrices are host-built 108x96 slab Toeplitz with replicate-edge
weights folded in, so every matmul contraction is K=108<=128, single pass.

Label remap (pure 32-entry LUT gather) is done on the host.
"""

import sys

for _p in ("/opt/trn_rl_repo",):
    if _p not in sys.path:
        sys.path.insert(0, _p)

import numpy as np
import ml_dtypes

import concourse.bass as bass
import concourse.mybir as mybir
import concourse.bacc as bacc
import concourse.tile as tile
from concourse.bass_utils import run_bass_kernel_spmd

F32 = mybir.dt.float32
BF16 = mybir.dt.bfloat16

B, C, D, H, W = 4, 1, 192, 192, 192
SMALL = 4
BIAS_STD = 0.7
MAX_SIGMA = 3.0
TRUNCATE = 4.0
K = 2 * int(TRUNCATE * MAX_SIGMA) + 1  # 25
P = K // 2  # 12
N_LABELS = 32
TABLE = 128

SLAB = 108          # folded halo slab rows (96 interior + 12, edges folded)
HW = H * W          # 36864
HB = 8              # h rows per phase-A chunk
NHB = H // HB       # 24
CH = HB * W         # 1536 cols per x chunk
QB = 512            # bias/exp psum grain
NQ = CH // QB       # 3
HQ = 4              # h rows per fused-D psum tile (4*96 = 384 cols)
DP = 2              # d' per phase-B step
NG = 96 // DP       # 48
IGB = 4             # g-steps per img staging DMA (8 d' = 1536 cols)

_CACHE = {}


def _lin_weights(n_in, n_out):
    pos = np.linspace(0.0, n_in - 1.0, n_out, dtype=np.float64)
    i0 = np.clip(np.floor(pos).astype(np.int64), 0, n_in - 2)
    f = pos - i0
    Wm = np.zeros((n_out, n_in), np.float64)
    r = np.arange(n_out)
    np.add.at(Wm, (r, i0), 1.0 - f)
    np.add.at(Wm, (r, i0 + 1), f)
    return Wm


def _gauss_kernels(sigma3):
    """sigma3 [3] -> [3, K] kernels exactly as reference."""
    ar = np.arange(K, dtype=np.float64) - K // 2
    out = np.zeros((3, K), np.float64)
    for i, sg in enumerate(sigma3):
        s = max(float(sg), 1e-3)
        g = np.exp(-0.5 * ar * ar / (s * s))
        g = g / g.sum()
        if float(sg) >= 0.01:
            out[i] = g
        else:
            out[i, K // 2] = 1.0
    return out


def _slab_mat(g, out0, slab0, n):
    """[SLAB, 96]: M[src - slab0, j] += g[t] with src = clip(out0+j+t-P, 0, n-1).

    Column j produces absolute output index out0+j from slab rows
    [slab0, slab0+SLAB); replicate-edge taps are folded into edge rows.
    """
    M = np.zeros((SLAB, 96), np.float64)
    for j in range(96):
        for t in range(K):
            src = min(max(out0 + j + t - P, 0), n - 1)
            M[src - slab0, j] += g[t]
    return M


def _build_program():
    nc = bacc.Bacc("TRN2", target_bir_lowering=False, debug=False)

    # ---- external inputs (per core) ----
    xbf_h = nc.dram_tensor("xbf", [SLAB, HW], BF16, kind="ExternalInput")
    sf_h = nc.dram_tensor("sf", [4, HW], BF16, kind="ExternalInput")
    wdt_h = nc.dram_tensor("wdt", [4, SLAB], BF16, kind="ExternalInput")
    gd_h = nc.dram_tensor("gd", [SLAB, 96], BF16, kind="ExternalInput")
    gwa_h = nc.dram_tensor("gwa", [SLAB, 96], BF16, kind="ExternalInput")
    gwb_h = nc.dram_tensor("gwb", [SLAB, 96], BF16, kind="ExternalInput")
    gha_h = nc.dram_tensor("gha", [SLAB, 96], BF16, kind="ExternalInput")
    ghb_h = nc.dram_tensor("ghb", [SLAB, 96], BF16, kind="ExternalInput")

    # ---- external outputs: [d'*192 + w', h'] ----
    img_h = nc.dram_tensor("img", [96 * W, H], BF16, kind="ExternalOutput")

    EXP = mybir.ActivationFunctionType.Exp
    MULT = mybir.AluOpType.mult

    with tile.TileContext(nc) as tc:
        with (
            tc.tile_pool(name="cst", bufs=1) as cst,
            tc.tile_pool(name="res", bufs=1) as res,
            tc.tile_pool(name="xcp", bufs=5) as xcp,
            tc.tile_pool(name="scp", bufs=5) as scp,
            tc.tile_pool(name="ebp", bufs=4) as ebp,
            tc.tile_pool(name="xbp", bufs=4) as xbp,
            tc.tile_pool(name="iap", bufs=3) as iap,
            tc.tile_pool(name="ps", bufs=3, space="PSUM") as psp,
            tc.tile_pool(name="ps2", bufs=5, space="PSUM") as psp2,
        ):
            # ---- constants to SBUF ----
            wdt = cst.tile([4, SLAB], BF16, name="wdt")
            nc.sync.dma_start(wdt[:], wdt_h.ap())
            gd = cst.tile([SLAB, 96], BF16, name="gd")
            nc.sync.dma_start(gd[:], gd_h.ap())
            gwa = cst.tile([SLAB, 96], BF16, name="gwa")
            gwb = cst.tile([SLAB, 96], BF16, name="gwb")
            gha = cst.tile([SLAB, 96], BF16, name="gha")
            ghb = cst.tile([SLAB, 96], BF16, name="ghb")

            # ---- full S operand (bias matmul rhs), loaded once ----
            scf = res.tile([4, HW], BF16, name="scf")
            nc.sync.dma_start(scf[:, 0:CH * 2], sf_h.ap()[:, 0:CH * 2])

            # ---- resident transposed volumes ----
            zwa = res.tile([SLAB, 96 * H], BF16, name="zwa")  # w 0..108 | (d', h)
            zwb = res.tile([SLAB, 96 * H], BF16, name="zwb")  # w 84..192 | (d', h)
            # zh tiles alias zw: fused_w(g) consumes zw block g right before
            # the zh haul overwrites the same block (WAR dep, range-tracked).
            zha = zwa  # h 0..108 | (d', w')
            zhb = zwb  # h 84..192 | (d', w')

            nzw = 0

            def haul_zw(dst, src):
                nonlocal nzw
                if False:
                    nc.scalar.copy(dst, src)
                else:
                    nc.vector.tensor_copy(dst, src)
                nzw += 1

            # ============ phase A: bias, exp, mult, fused D-blur+T ========
            def front_end(hb):
                c0 = hb * CH
                xc = xcp.tile([SLAB, CH], BF16, tag="xc")
                nc.sync.dma_start(xc[:], xbf_h.ap()[:, c0:c0 + CH])
                eb = ebp.tile([SLAB, CH], BF16, tag="eb")
                xb = xbp.tile([SLAB, CH], BF16, tag="xb")
                for q in range(NQ):
                    sl = slice(q * QB, (q + 1) * QB)
                    psb = psp.tile([SLAB, QB], F32, tag="psb")
                    nc.tensor.matmul(psb[:], wdt[:], scf[:, c0 + q * QB: c0 + (q + 1) * QB],
                                     start=True, stop=True)
                    nc.scalar.activation(eb[:, sl], psb[:], EXP)
                    if q % 3 == 0:
                        nc.vector.tensor_tensor(xb[:, sl], xc[:, sl], eb[:, sl], MULT)
                    else:
                        nc.gpsimd.tensor_tensor(xb[:, sl], xc[:, sl], eb[:, sl], MULT)
                return xb

            def fused_d(hb, xb):
                # stationary = xb w-slab cols of one h row; moving = gd.
                # psum [w-slab, d'] batched over HQ h rows.
                for hq in range(HB // HQ):
                    psa = psp2.tile([SLAB, HQ * 96], F32, tag="psa")
                    psb2 = psp2.tile([SLAB, HQ * 96], F32, tag="psa")
                    for t in range(HQ):
                        hl = hq * HQ + t
                        nc.tensor.matmul(
                            psa[:, t * 96:(t + 1) * 96],
                            xb[:, hl * W: hl * W + SLAB], gd[:],
                            start=True, stop=True)
                        nc.tensor.matmul(
                            psb2[:, t * 96:(t + 1) * 96],
                            xb[:, hl * W + 84: hl * W + 192], gd[:],
                            start=True, stop=True)
                    # psum cols (h-local, d') -> zw cols d'*192 + h
                    # late chunks: ACT's exp queue has drained, give it the
                    # second haul of each pair
                    h0 = hb * HB + hq * HQ
                    for i, (ps_, zw_) in enumerate(((psa, zwa), (psb2, zwb))):
                        dst = zw_[:].rearrange("p (d h) -> p h d", h=H)[:, h0:h0 + HQ, :]
                        s_ = ps_[:].rearrange("p (t d) -> p t d", t=HQ)
                        if hb >= 18 and i == 1:
                            nc.scalar.copy(dst, s_)
                        else:
                            nc.vector.tensor_copy(dst, s_)

            prev = None
            for hb in range(NHB):
                if prev is not None:
                    fused_d(hb - 1, prev)
                prev = front_end(hb)
                if hb == 0:
                    nc.sync.dma_start(scf[:, CH * 2:HW // 2],
                                      sf_h.ap()[:, CH * 2:HW // 2])
                if hb == 10:
                    nc.sync.dma_start(scf[:, HW // 2:HW],
                                      sf_h.ap()[:, HW // 2:HW])
                if hb == 23:
                    # phase-B constants: load after the pipeline is primed
                    nc.sync.dma_start(gwa[:], gwa_h.ap())
                    nc.sync.dma_start(gwb[:], gwb_h.ap())
                    nc.sync.dma_start(gha[:], gha_h.ap())
                    nc.sync.dma_start(ghb[:], ghb_h.ap())
            fused_d(NHB - 1, prev)

            # ========= phase B: fused W-blur+T, fused H-blur, img out ==
            nzh = 0

            def fused_w(g):
                psta = psp2.tile([SLAB, DP * W], F32, tag="psa")
                pstb = psp2.tile([SLAB, DP * W], F32, tag="psa")
                for dl in range(DP):
                    dcol = (DP * g + dl) * H
                    nc.tensor.matmul(
                        psta[:, dl * W + 0: dl * W + 96],
                        zwa[:, dcol + 0: dcol + SLAB], gwa[:],
                        start=True, stop=True)
                    nc.tensor.matmul(
                        psta[:, dl * W + 96: dl * W + 192],
                        zwb[:, dcol + 0: dcol + SLAB], gwb[:],
                        start=True, stop=True)
                    nc.tensor.matmul(
                        pstb[:, dl * W + 0: dl * W + 96],
                        zwa[:, dcol + 84: dcol + 192], gwa[:],
                        start=True, stop=True)
                    nc.tensor.matmul(
                        pstb[:, dl * W + 96: dl * W + 192],
                        zwb[:, dcol + 84: dcol + 192], gwb[:],
                        start=True, stop=True)
                nonlocal nzh
                gsl = slice(g * DP * W, (g + 1) * DP * W)
                for (z_, ps_) in ((zha, psta), (zhb, pstb)):
                    if nzh % 2 == 1:
                        nc.scalar.copy(z_[:, gsl], ps_[:])
                    else:
                        nc.vector.tensor_copy(z_[:, gsl], ps_[:])
                    nzh += 1

            # fused H: stationary = zh dw-cols (128 at a time); moving = gh
            # slab. psum [dw, (cc, h')]; two chunks per psum tile.
            ist = None

            def fused_h(hp):
                nonlocal ist
                psh = psp.tile([128, 2 * H], F32, tag="psb")
                for cc in range(2):
                    cb = 256 * hp + 128 * cc
                    nc.tensor.matmul(
                        psh[:, cc * H + 0: cc * H + 96],
                        zha[:, cb:cb + 128], gha[:], start=True, stop=True)
                    nc.tensor.matmul(
                        psh[:, cc * H + 96: cc * H + 192],
                        zhb[:, cb:cb + 128], ghb[:], start=True, stop=True)
                if hp % 8 == 0:
                    ist = iap.tile([128, 16 * H], BF16, tag="ist")
                isl = slice((hp % 8) * 2 * H, (hp % 8 + 1) * 2 * H)
                if hp % 2 == 1:
                    nc.scalar.copy(ist[:, isl], psh[:])
                else:
                    nc.vector.tensor_copy(ist[:, isl], psh[:])
                if hp % 8 == 7:
                    # partition-major rows: dram row = p*144 + chunk
                    c0 = (hp // 8) * 16
                    nc.sync.dma_start(
                        bass.AP(img_h, c0 * H,
                                [[144 * H, 128], [H, 16], [1, H]]),
                        ist[:])

            NHP = 96 * W // 256  # 72 fused-H tiles
            hptr = 0
            for g in range(NG):
                fused_w(g)
                while (hptr + 1) * 256 <= 384 * g:
                    fused_h(hptr)
                    hptr += 1
            while hptr < NHP:
                fused_h(hptr)
                hptr += 1
    nc.compile()
    return nc


def _host_prep(x, small_bias, sigma01):
    Wd = _lin_weights(SMALL, D)
    Whm = _lin_weights(SMALL, H)
    Wwm = _lin_weights(SMALL, W)

    in_maps = []
    for c in range(8):
        b, half = c // 2, c % 2
        slab0 = 0 if half == 0 else 84
        d0 = 96 * half

        xs = np.ascontiguousarray(
            np.asarray(x[b, 0, slab0:slab0 + SLAB], np.float32)
        ).reshape(SLAB, HW).astype(ml_dtypes.bfloat16)

        sm = np.asarray(small_bias[b, 0], np.float64) * BIAS_STD
        S = np.einsum("xyz,hy,wz->xhw", sm, Whm, Wwm).reshape(4, HW)

        g3 = _gauss_kernels(np.asarray(sigma01[b], np.float64) * MAX_SIGMA)
        Gd = _slab_mat(g3[0], d0, slab0, D)
        Gwa = _slab_mat(g3[2], 0, 0, W)
        Gwb = _slab_mat(g3[2], 96, 84, W)
        Gha = _slab_mat(g3[1], 0, 0, H)
        Ghb = _slab_mat(g3[1], 96, 84, H)

        in_maps.append({
            "xbf": xs,
            "sf": S.astype(ml_dtypes.bfloat16),
            "wdt": np.ascontiguousarray(Wd[slab0:slab0 + SLAB].T).astype(ml_dtypes.bfloat16),
            "gd": Gd.astype(ml_dtypes.bfloat16),
            "gwa": Gwa.astype(ml_dtypes.bfloat16),
            "gwb": Gwb.astype(ml_dtypes.bfloat16),
            "gha": Gha.astype(ml_dtypes.bfloat16),
            "ghb": Ghb.astype(ml_dtypes.bfloat16),
        })
    return in_maps


def kernel(x, small_bias, sigma01, labels, source_values, dest_values):
    if "nc" not in _CACHE:
        _CACHE["nc"] = _build_program()
    nc = _CACHE["nc"]

    in_maps = _host_prep(x, small_bias, sigma01)
    res = run_bass_kernel_spmd(nc, in_maps, core_ids=list(range(8)))

    img = np.empty((B, C, D, H, W), np.float32)
    for c in range(8):
        b, half = c // 2, c % 2
        d0 = 96 * half
        r = np.asarray(res.results[c]["img"], ml_dtypes.bfloat16)
        # rows are partition-major: row = p*144 + chunk, dw = chunk*128 + p
        r = r.reshape(128, 144, H).transpose(1, 0, 2).reshape(96, W, H)
        # [d', w', h'] -> [d', h', w']
        img[b, 0, d0:d0 + 96] = r.transpose(0, 2, 1).astype(np.float32)

    mapping = np.zeros(TABLE, np.int32)
    mapping[np.asarray(source_values, np.int64)] = (
        np.asarray(dest_values, np.int64).astype(np.int32))
    labels_out = mapping[np.asarray(labels, np.int64)]
    return img, labels_out


# revision 124
# speedup vs baseline: 1.0700x; 1.0058x over previous
"""Trainium2 Bass kernel for nn_BrainGeneratorModel (bias-field corrupt + per-sample
separable Gaussian blur), 8-core data parallel.

Sharding: 8 cores = (sample b in 0..3) x (D-half in 0..1). Each core processes a
[108-slab d, H=192, W=192] subvolume (96 interior d' + edge-folded halo) of one
sample.

Per-core pipeline, fully SBUF-resident (no DRAM scratch). Blur+transpose stages
are fused into single data-stationary matmuls: matmul(out, lhsT=DATA, rhs=G)
computes the blur with the data's free dim landing on output partitions, so no
separate PE transpose passes are needed:
  phase A (stream h): bias matmul (K=4) -> exp (ACT) -> x*e (Pool, bf16)
    -> fused D-blur+transpose (stationary = x*e w-slab cols, moving = Gd)
       -> psum [w-slab, d'] -> zw resident tiles, layout (d', h)
  phase B (stream d'): fused W-blur+transpose (stationary = zw h-slab cols,
       moving = Gw slab) -> psum [h-slab, w'] -> zh tiles, layout (d', w')
    -> classic H-blur matmul -> img out (bf16)
All blur matrices are host-built 108x96 slab Toeplitz with replicate-edge
weights folded in, so every matmul contraction is K=108<=128, single pass.

Label remap (pure 32-entry LUT gather) is done on the host.
"""

import sys

for _p in ("/opt/trn_rl_repo",):
    if _p not in sys.path:
        sys.path.insert(0, _p)

import numpy as np
import ml_dtypes

import concourse.bass as bass
import concourse.mybir as mybir
import concourse.bacc as bacc
import concourse.tile as tile
from concourse.bass_utils import run_bass_kernel_spmd

F32 = mybir.dt.float32
BF16 = mybir.dt.bfloat16

B, C, D, H, W = 4, 1, 192, 192, 192
SMALL = 4
BIAS_STD = 0.7
MAX_SIGMA = 3.0
TRUNCATE = 4.0
K = 2 * int(TRUNCATE * MAX_SIGMA) + 1  # 25
P = K // 2  # 12
N_LABELS = 32
TABLE = 128

SLAB = 108          # folded halo slab rows (96 interior + 12, edges folded)
HW = H * W          # 36864
HB = 8              # h rows per phase-A chunk
NHB = H // HB       # 24
CH = HB * W         # 1536 cols per x chunk
QB = 512            # bias/exp psum grain
NQ = CH // QB       # 3
HQ = 4              # h rows per fused-D psum tile (4*96 = 384 cols)
DP = 2              # d' per phase-B step
NG = 96 // DP       # 48
IGB = 4             # g-steps per img staging DMA (8 d' = 1536 cols)

_CACHE = {}


def _lin_weights(n_in, n_out):
    pos = np.linspace(0.0, n_in - 1.0, n_out, dtype=np.float64)
    i0 = np.clip(np.floor(pos).astype(np.int64), 0, n_in - 2)
    f = pos - i0
    Wm = np.zeros((n_out, n_in), np.float64)
    r = np.arange(n_out)
    np.add.at(Wm, (r, i0), 1.0 - f)
    np.add.at(Wm, (r, i0 + 1), f)
    return Wm


def _gauss_kernels(sigma3):
    """sigma3 [3] -> [3, K] kernels exactly as reference."""
    ar = np.arange(K, dtype=np.float64) - K // 2
    out = np.zeros((3, K), np.float64)
    for i, sg in enumerate(sigma3):
        s = max(float(sg), 1e-3)
        g = np.exp(-0.5 * ar * ar / (s * s))
        g = g / g.sum()
        if float(sg) >= 0.01:
            out[i] = g
        else:
            out[i, K // 2] = 1.0
    return out


def _slab_mat(g, out0, slab0, n):
    """[SLAB, 96]: M[src - slab0, j] += g[t] with src = clip(out0+j+t-P, 0, n-1).

    Column j produces absolute output index out0+j from slab rows
    [slab0, slab0+SLAB); replicate-edge taps are folded into edge rows.
    """
    M = np.zeros((SLAB, 96), np.float64)
    for j in range(96):
        for t in range(K):
            src = min(max(out0 + j + t - P, 0), n - 1)
            M[src - slab0, j] += g[t]
    return M


def _build_program():
    nc = bacc.Bacc("TRN2", target_bir_lowering=False, debug=False)

    # ---- external inputs (per core) ----
    xbf_h = nc.dram_tensor("xbf", [SLAB, HW], BF16, kind="ExternalInput")
    sf_h = nc.dram_tensor("sf", [4, HW], BF16, kind="ExternalInput")
    wdt_h = nc.dram_tensor("wdt", [4, SLAB], BF16, kind="ExternalInput")
    gd_h = nc.dram_tensor("gd", [SLAB, 96], BF16, kind="ExternalInput")
    gwa_h = nc.dram_tensor("gwa", [SLAB, 96], BF16, kind="ExternalInput")
    gwb_h = nc.dram_tensor("gwb", [SLAB, 96], BF16, kind="ExternalInput")
    gha_h = nc.dram_tensor("gha", [SLAB, 96], BF16, kind="ExternalInput")
    ghb_h = nc.dram_tensor("ghb", [SLAB, 96], BF16, kind="ExternalInput")

    # ---- external outputs: [d'*192 + w', h'] ----
    img_h = nc.dram_tensor("img", [96 * W, H], BF16, kind="ExternalOutput")

    EXP = mybir.ActivationFunctionType.Exp
    MULT = mybir.AluOpType.mult

    with tile.TileContext(nc) as tc:
        with (
            tc.tile_pool(name="cst", bufs=1) as cst,
            tc.tile_pool(name="res", bufs=1) as res,
            tc.tile_pool(name="xcp", bufs=5) as xcp,
            tc.tile_pool(name="scp", bufs=5) as scp,
            tc.tile_pool(name="ebp", bufs=4) as ebp,
            tc.tile_pool(name="xbp", bufs=4) as xbp,
            tc.tile_pool(name="iap", bufs=3) as iap,
            tc.tile_pool(name="ps", bufs=3, space="PSUM") as psp,
            tc.tile_pool(name="ps2", bufs=5, space="PSUM") as psp2,
        ):
            # ---- constants to SBUF ----
            wdt = cst.tile([4, SLAB], BF16, name="wdt")
            nc.sync.dma_start(wdt[:], wdt_h.ap())
            gd = cst.tile([SLAB, 96], BF16, name="gd")
            nc.sync.dma_start(gd[:], gd_h.ap())
            gwa = cst.tile([SLAB, 96], BF16, name="gwa")
            gwb = cst.tile([SLAB, 96], BF16, name="gwb")
            gha = cst.tile([SLAB, 96], BF16, name="gha")
            ghb = cst.tile([SLAB, 96], BF16, name="ghb")

            # ---- full S operand (bias matmul rhs), loaded once ----
            scf = res.tile([4, HW], BF16, name="scf")
            nc.sync.dma_start(scf[:, 0:CH * 2], sf_h.ap()[:, 0:CH * 2])

            # ---- resident transposed volumes ----
            zwa = res.tile([SLAB, 96 * H], BF16, name="zwa")  # w 0..108 | (d', h)
            zwb = res.tile([SLAB, 96 * H], BF16, name="zwb")  # w 84..192 | (d', h)
            # zh tiles alias zw: fused_w(g) consumes zw block g right before
            # the zh haul overwrites the same block (WAR dep, range-tracked).
            zha = zwa  # h 0..108 | (d', w')
            zhb = zwb  # h 84..192 | (d', w')

            nzw = 0

            def haul_zw(dst, src):
                nonlocal nzw
                if False:
                    nc.scalar.copy(dst, src)
                else:
                    nc.vector.tensor_copy(dst, src)
                nzw += 1

            # ============ phase A: bias, exp, mult, fused D-blur+T ========
            def front_end(hb):
                c0 = hb * CH
                xc = xcp.tile([SLAB, CH], BF16, tag="xc")
                nc.sync.dma_start(xc[:], xbf_h.ap()[:, c0:c0 + CH])
                eb = ebp.tile([SLAB, CH], BF16, tag="eb")
                xb = xbp.tile([SLAB, CH], BF16, tag="xb")
                for q in range(NQ):
                    sl = slice(q * QB, (q + 1) * QB)
                    psb = psp.tile([SLAB, QB], F32, tag="psb")
                    nc.tensor.matmul(psb[:], wdt[:], scf[:, c0 + q * QB: c0 + (q + 1) * QB],
                                     start=True, stop=True)
                    nc.scalar.activation(eb[:, sl], psb[:], EXP)
                    if q % 3 == 0 or (hb >= 22 and q == 1):
                        nc.vector.tensor_tensor(xb[:, sl], xc[:, sl], eb[:, sl], MULT)
                    else:
                        nc.gpsimd.tensor_tensor(xb[:, sl], xc[:, sl], eb[:, sl], MULT)
                return xb

            def fused_d(hb, xb):
                # stationary = xb w-slab cols of one h row; moving = gd.
                # psum [w-slab, d'] batched over HQ h rows.
                for hq in range(HB // HQ):
                    psa = psp2.tile([SLAB, HQ * 96], F32, tag="psa")
                    psb2 = psp2.tile([SLAB, HQ * 96], F32, tag="psa")
                    for t in range(HQ):
                        hl = hq * HQ + t
                        nc.tensor.matmul(
                            psa[:, t * 96:(t + 1) * 96],
                            xb[:, hl * W: hl * W + SLAB], gd[:],
                            start=True, stop=True)
                        nc.tensor.matmul(
                            psb2[:, t * 96:(t + 1) * 96],
                            xb[:, hl * W + 84: hl * W + 192], gd[:],
                            start=True, stop=True)
                    # psum cols (h-local, d') -> zw cols d'*192 + h
                    # late chunks: ACT's exp queue has drained, give it the
                    # second haul of each pair
                    h0 = hb * HB + hq * HQ
                    for i, (ps_, zw_) in enumerate(((psa, zwa), (psb2, zwb))):
                        dst = zw_[:].rearrange("p (d h) -> p h d", h=H)[:, h0:h0 + HQ, :]
                        s_ = ps_[:].rearrange("p (t d) -> p t d", t=HQ)
                        if hb >= 18 and i == 1:
                            nc.scalar.copy(dst, s_)
                        else:
                            nc.vector.tensor_copy(dst, s_)

            prev = None
            for hb in range(NHB):
                if prev is not None:
                    fused_d(hb - 1, prev)
                prev = front_end(hb)
                if hb == 0:
                    nc.sync.dma_start(scf[:, CH * 2:HW // 2],
                                      sf_h.ap()[:, CH * 2:HW // 2])
                if hb == 10:
                    nc.sync.dma_start(scf[:, HW // 2:HW],
                                      sf_h.ap()[:, HW // 2:HW])
                if hb == 23:
                    # phase-B constants: load after the pipeline is primed
                    nc.sync.dma_start(gwa[:], gwa_h.ap())
                    nc.sync.dma_start(gwb[:], gwb_h.ap())
                    nc.sync.dma_start(gha[:], gha_h.ap())
                    nc.sync.dma_start(ghb[:], ghb_h.ap())
            fused_d(NHB - 1, prev)

            # ========= phase B: fused W-blur+T, fused H-blur, img out ==
            nzh = 0

            def fused_w(g):
                psta = psp2.tile([SLAB, DP * W], F32, tag="psa")
                pstb = psp2.tile([SLAB, DP * W], F32, tag="psa")
                for dl in range(DP):
                    dcol = (DP * g + dl) * H
                    nc.tensor.matmul(
                        psta[:, dl * W + 0: dl * W + 96],
                        zwa[:, dcol + 0: dcol + SLAB], gwa[:],
                        start=True, stop=True)
                    nc.tensor.matmul(
                        psta[:, dl * W + 96: dl * W + 192],
                        zwb[:, dcol + 0: dcol + SLAB], gwb[:],
                        start=True, stop=True)
                    nc.tensor.matmul(
                        pstb[:, dl * W + 0: dl * W + 96],
                        zwa[:, dcol + 84: dcol + 192], gwa[:],
                        start=True, stop=True)
                    nc.tensor.matmul(
                        pstb[:, dl * W + 96: dl * W + 192],
                        zwb[:, dcol + 84: dcol + 192], gwb[:],
                        start=True, stop=True)
                nonlocal nzh
                gsl = slice(g * DP * W, (g + 1) * DP * W)
                for (z_, ps_) in ((zha, psta), (zhb, pstb)):
                    if nzh % 2 == 1:
                        nc.scalar.copy(z_[:, gsl], ps_[:])
                    else:
                        nc.vector.tensor_copy(z_[:, gsl], ps_[:])
                    nzh += 1

            # fused H: stationary = zh dw-cols (128 at a time); moving = gh
            # slab. psum [dw, (cc, h')]; two chunks per psum tile.
            ist = None

            def fused_h(hp):
                nonlocal ist
                psh = psp.tile([128, 2 * H], F32, tag="psb")
                for cc in range(2):
                    cb = 256 * hp + 128 * cc
                    nc.tensor.matmul(
                        psh[:, cc * H + 0: cc * H + 96],
                        zha[:, cb:cb + 128], gha[:], start=True, stop=True)
                    nc.tensor.matmul(
                        psh[:, cc * H + 96: cc * H + 192],
                        zhb[:, cb:cb + 128], ghb[:], start=True, stop=True)
                if hp % 8 == 0:
                    ist = iap.tile([128, 16 * H], BF16, tag="ist")
                isl = slice((hp % 8) * 2 * H, (hp % 8 + 1) * 2 * H)
                if hp % 2 == 1:
                    nc.scalar.copy(ist[:, isl], psh[:])
                else:
                    nc.vector.tensor_copy(ist[:, isl], psh[:])
                if hp % 8 == 7:
                    # partition-major rows: dram row = p*144 + chunk
                    c0 = (hp // 8) * 16
                    nc.sync.dma_start(
                        bass.AP(img_h, c0 * H,
                                [[144 * H, 128], [H, 16], [1, H]]),
                        ist[:])

            NHP = 96 * W // 256  # 72 fused-H tiles
            hptr = 0
            for g in range(NG):
                fused_w(g)
                while (hptr + 1) * 256 <= 384 * g:
                    fused_h(hptr)
                    hptr += 1
            while hptr < NHP:
                fused_h(hptr)
                hptr += 1
    nc.compile()
    return nc


def _host_prep(x, small_bias, sigma01):
    Wd = _lin_weights(SMALL, D)
    Whm = _lin_weights(SMALL, H)
    Wwm = _lin_weights(SMALL, W)

    in_maps = []
    for c in range(8):
        b, half = c // 2, c % 2
        slab0 = 0 if half == 0 else 84
        d0 = 96 * half

        xs = np.ascontiguousarray(
            np.asarray(x[b, 0, slab0:slab0 + SLAB], np.float32)
        ).reshape(SLAB, HW).astype(ml_dtypes.bfloat16)

        sm = np.asarray(small_bias[b, 0], np.float64) * BIAS_STD
        S = np.einsum("xyz,hy,wz->xhw", sm, Whm, Wwm).reshape(4, HW)

        g3 = _gauss_kernels(np.asarray(sigma01[b], np.float64) * MAX_SIGMA)
        Gd = _slab_mat(g3[0], d0, slab0, D)
        Gwa = _slab_mat(g3[2], 0, 0, W)
        Gwb = _slab_mat(g3[2], 96, 84, W)
        Gha = _slab_mat(g3[1], 0, 0, H)
        Ghb = _slab_mat(g3[1], 96, 84, H)

        in_maps.append({
            "xbf": xs,
            "sf": S.astype(ml_dtypes.bfloat16),
            "wdt": np.ascontiguousarray(Wd[slab0:slab0 + SLAB].T).astype(ml_dtypes.bfloat16),
            "gd": Gd.astype(ml_dtypes.bfloat16),
            "gwa": Gwa.astype(ml_dtypes.bfloat16),
            "gwb": Gwb.astype(ml_dtypes.bfloat16),
            "gha": Gha.astype(ml_dtypes.bfloat16),
            "ghb": Ghb.astype(ml_dtypes.bfloat16),
        })
    return in_maps


def kernel(x, small_bias, sigma01, labels, source_values, dest_values):
    if "nc" not in _CACHE:
        _CACHE["nc"] = _build_program()
    nc = _CACHE["nc"]

    in_maps = _host_prep(x, small_bias, sigma01)
    res = run_bass_kernel_spmd(nc, in_maps, core_ids=list(range(8)))

    img = np.empty((B, C, D, H, W), np.float32)
    for c in range(8):
        b, half = c // 2, c % 2
        d0 = 96 * half
        r = np.asarray(res.results[c]["img"], ml_dtypes.bfloat16)
        # rows are partition-major: row = p*144 + chunk, dw = chunk*128 + p
        r = r.reshape(128, 144, H).transpose(1, 0, 2).reshape(96, W, H)
        # [d', w', h'] -> [d', h', w']
        img[b, 0, d0:d0 + 96] = r.transpose(0, 2, 1).astype(np.float32)

    mapping = np.zeros(TABLE, np.int32)
    mapping[np.asarray(source_values, np.int64)] = (
        np.asarray(dest_values, np.int64).astype(np.int32))
    labels_out = mapping[np.asarray(labels, np.int64)]
    return img, labels_out
